# revision 7
# baseline (speedup 1.0000x reference)
"""Conv2d 3x3 (stride 1, pad 1) forward on 8 Trainium2 NeuronCores.

Problem: x (32,32,128,128) f32, kernel (64,32,3,3), bias (64)
         -> out (32,64,128,128).  Data-parallel: 4 images per core.

v6 design (at the PE feed-rate roofline):
  - All off-chip traffic is fp16 (tolerance 2e-2; fp16 conv err ~3e-4).
    x is cast + zero-padded on the host to [128, 130, 130] f16 per core
    so the load is 4.1 MiB of contiguous 8KB descriptors; the output is
    f16 in a device-friendly layout (8 MiB), un-permuted on the host.
  - Compute: per round 4 output rows/image; 9 shifted matmuls accumulate
    in PSUM; 4 concurrent streams on the 4 PE row groups (the SBUF feed
    rate of 1 elem/partition/cycle makes this the max useful rate for
    Cout=64).  Images 0,1 output on column groups 0-1, images 2,3 on
    column groups 2-3, so one PSUM bank holds two images and the drains
    are full-width: per round one ScalarE activation (+bias, ->f16) and
    one VectorE tensor_scalar_add.
  - Head: tiny first x chunk [0,19) + 7 warm-up matmuls on zeros so the
    HAM clock gate is released and the PE is hot when real data lands.
  - Tail: stores grouped 4+4+...+2+1+1 rounds so the final store chain
    after the last matmul is short.
"""
import sys
sys.path.insert(0, '/opt/trn_rl_repo')
import numpy as np

B, Cin, H, W = 32, 32, 128, 128
Cout, KH, KW = 64, 3, 3
NCORES = 8
BPC = B // NCORES          # images per core
Hp, Wp = H + 2, W + 2
NTAP = KH * KW
ROWS = 4                   # output rows per round
NROUND = H // ROWS
NWARM = 7                  # PE warm-up matmuls
GROUPS = [(0, 4), (4, 8), (8, 12), (12, 16), (16, 20), (20, 24),
          (24, 28), (28, 30), (30, 31), (31, 32)]

_cache = {}


def _build_program():
    from concourse import bacc
    import concourse.mybir as mybir
    from concourse.tile import TileContext

    f32 = mybir.dt.float32
    f16 = mybir.dt.float16
    Act = mybir.ActivationFunctionType

    nc = bacc.Bacc("TRN2", target_bir_lowering=False, debug=False,
                   num_devices=NCORES)
    x_ext = nc.declare_dram_parameter("x", [128, Hp, Wp], f16, isOutput=False)
    w_ext = nc.declare_dram_parameter("w", [128, NTAP, Cout], f16,
                                      isOutput=False)
    b_ext = nc.declare_dram_parameter("b", [128, 1], f32, isOutput=False)
    # out[p, k, pair, rho, w]: p = 64*ph + co; image = 2*ph + pair;
    # h = ROWS*k + rho
    out_ext = nc.declare_dram_parameter(
        "out", [128, NROUND, 2, ROWS, W], f16, isOutput=True)

    with TileContext(nc) as tc:
        with tc.tile_pool(name="xp", bufs=1) as xpool, \
             tc.tile_pool(name="const", bufs=1) as cpool, \
             tc.tile_pool(name="stage", bufs=3) as opool, \
             tc.tile_pool(name="psum", bufs=8, space="PSUM") as ppool:

            xp = xpool.tile([128, Hp, Wp], f16)
            wt = cpool.tile([128, NTAP, Cout], f16)
            bt = cpool.tile([128, 1], f32)
            zt = cpool.tile([32, 512], f16)

            nc.sync.dma_start(out=wt[:], in_=w_ext[:])
            nc.sync.dma_start(out=bt[:], in_=b_ext[:])

            # x load: tiny first chunk so round 0 starts promptly
            bounds = [0, 11, 43, 75, 107, Hp]
            for g in range(len(bounds) - 1):
                a, b = bounds[g], bounds[g + 1]
                nc.sync.dma_start(out=xp[:, a:b, :], in_=x_ext[:, a:b, :])

            # PE warm-up: release the HAM clock gate during the x load.
            nc.vector.memset(zt[:].bitcast(f32), 0.0)
            pw = ppool.tile([128, ROWS, W], f32, tag="ps", name="warm")
            for i in range(NWARM):
                nc.tensor.matmul(pw[0:64], zt[0:32, 0:64], zt[0:32, :],
                                 start=True, stop=True, tile_position=(0, 0),
                                 skip_group_check=True)

            for (k0, k1) in GROUPS:
                ost = opool.tile([128, k1 - k0, 2, ROWS, W], f16, tag="ost")
                for k in range(k0, k1):
                    h0 = k * ROWS
                    slot = k - k0
                    ps0 = ppool.tile([128, ROWS, W], f32, tag="ps",
                                     name=f"ps{k}_0")   # imgs 0,2
                    ps1 = ppool.tile([128, ROWS, W], f32, tag="ps",
                                     name=f"ps{k}_1")   # imgs 1,3
                    for t in range(NTAP):
                        kh, kw = divmod(t, 3)
                        st, sp = (t == 0), (t == NTAP - 1)
                        rows = xp[:, h0 + kh:h0 + kh + ROWS, kw:kw + W]
                        nc.tensor.matmul(ps0[0:64], wt[0:32, t, :],
                                         rows[0:32], start=st, stop=sp,
                                         tile_position=(0, 0))
                        nc.tensor.matmul(ps1[0:64], wt[32:64, t, :],
                                         rows[32:64], start=st, stop=sp,
                                         tile_position=(32, 0))
                        nc.tensor.matmul(ps0[64:128], wt[64:96, t, :],
                                         rows[64:96], start=st, stop=sp,
                                         tile_position=(64, 64))
                        nc.tensor.matmul(ps1[64:128], wt[96:128, t, :],
                                         rows[96:128], start=st, stop=sp,
                                         tile_position=(96, 64))
                    # full-width drains: +bias, cast f32->f16
                    nc.scalar.activation(ost[:, slot, 0, :, :], ps0[:, :, :],
                                         Act.Identity, bias=bt[:, :])
                    nc.vector.tensor_scalar_add(ost[:, slot, 1, :, :],
                                                ps1[:, :, :], bt[:, :])
                if k1 - k0 > 1:
                    nc.sync.dma_start(out=out_ext[:, k0:k1], in_=ost[:])
                else:
                    # final rounds: store each pair as soon as its drain lands
                    nc.sync.dma_start(out=out_ext[:, k0:k1, 0],
                                      in_=ost[:, :, 0])
                    nc.sync.dma_start(out=out_ext[:, k0:k1, 1],
                                      in_=ost[:, :, 1])

    nc.compile()
    return nc


def _get_program():
    if "nc" not in _cache:
        _cache["nc"] = _build_program()
    return _cache["nc"]


def _prep_inputs(x, kernel, bias):
    # weights: (Cout, Cin, KH, KW) -> [ci, tap, co], replicated on the
    # 4 PE row groups
    w = np.transpose(kernel.reshape(Cout, Cin, NTAP), (1, 2, 0))
    w = np.ascontiguousarray(np.tile(w, (4, 1, 1))).astype(np.float16)
    b = np.ascontiguousarray(
        np.tile(bias.astype(np.float32), 2)[:, None])
    x16 = x.astype(np.float16)
    in_maps = []
    for c in range(NCORES):
        xs = np.zeros((128, Hp, Wp), dtype=np.float16)
        xs[:, 1:1 + H, 1:1 + W] = x16[c * BPC:(c + 1) * BPC].reshape(
            BPC * Cin, H, W)
        in_maps.append({"x": xs, "w": w, "b": b})
    return in_maps


def _unshard(res):
    outs = []
    for c in range(NCORES):
        a = res.results[c]["out"]          # [128, NROUND, 2, ROWS, W] f16
        a = a.reshape(2, Cout, NROUND, 2, ROWS, W)   # [ph, co, k, pair, rho, w]
        a = np.transpose(a, (0, 3, 1, 2, 4, 5))      # [ph, pair, co, k, rho, w]
        outs.append(a.reshape(BPC, Cout, H, W))
    return np.concatenate(outs, axis=0).astype(np.float32)


def _run(inputs, trace=False):
    from concourse.bass_utils import run_bass_kernel_spmd
    nc = _get_program()
    in_maps = _prep_inputs(inputs["x"], inputs["kernel"], inputs["bias"])
    res = run_bass_kernel_spmd(nc, in_maps, list(range(NCORES)), trace=trace)
    return _unshard(res), res


def kernel(**inputs):
    out, _ = _run(inputs, trace=False)
    return out


# revision 9
# speedup vs baseline: 1.0181x; 1.0181x over previous
"""Conv2d 3x3 (stride 1, pad 1) forward on 8 Trainium2 NeuronCores.

Problem: x (32,32,128,128) f32, kernel (64,32,3,3), bias (64)
         -> out (32,64,128,128).  Data-parallel: 4 images per core.

v6 design (at the PE feed-rate roofline):
  - All off-chip traffic is fp16 (tolerance 2e-2; fp16 conv err ~3e-4).
    x is cast + zero-padded on the host to [128, 130, 130] f16 per core
    so the load is 4.1 MiB of contiguous 8KB descriptors; the output is
    f16 in a device-friendly layout (8 MiB), un-permuted on the host.
  - Compute: per round 4 output rows/image; 9 shifted matmuls accumulate
    in PSUM; 4 concurrent streams on the 4 PE row groups (the SBUF feed
    rate of 1 elem/partition/cycle makes this the max useful rate for
    Cout=64).  Images 0,1 output on column groups 0-1, images 2,3 on
    column groups 2-3, so one PSUM bank holds two images and the drains
    are full-width: per round one ScalarE activation (+bias, ->f16) and
    one VectorE tensor_scalar_add.
  - Head: tiny first x chunk [0,19) + 7 warm-up matmuls on zeros so the
    HAM clock gate is released and the PE is hot when real data lands.
  - Tail: stores grouped 4+4+...+2+1+1 rounds so the final store chain
    after the last matmul is short.
"""
import sys
sys.path.insert(0, '/opt/trn_rl_repo')
import numpy as np

B, Cin, H, W = 32, 32, 128, 128
Cout, KH, KW = 64, 3, 3
NCORES = 8
BPC = B // NCORES          # images per core
Hp, Wp = H + 2, W + 2
NTAP = KH * KW
ROWS = 4                   # output rows per round
NROUND = H // ROWS
NWARM = 10                 # PE warm-up matmuls
GROUPS = [(0, 4), (4, 8), (8, 12), (12, 16), (16, 20), (20, 24),
          (24, 28), (28, 30), (30, 31), (31, 32)]

_cache = {}


def _build_program():
    from concourse import bacc
    import concourse.mybir as mybir
    from concourse.tile import TileContext

    f32 = mybir.dt.float32
    f16 = mybir.dt.float16
    Act = mybir.ActivationFunctionType

    nc = bacc.Bacc("TRN2", target_bir_lowering=False, debug=False,
                   num_devices=NCORES)
    x_ext = nc.declare_dram_parameter("x", [128, Hp, Wp], f16, isOutput=False)
    w_ext = nc.declare_dram_parameter("w", [128, NTAP, Cout], f16,
                                      isOutput=False)
    b_ext = nc.declare_dram_parameter("b", [128, 1], f32, isOutput=False)
    # out[p, k, pair, rho, w]: p = 64*ph + co; image = 2*ph + pair;
    # h = ROWS*k + rho
    out_ext = nc.declare_dram_parameter(
        "out", [128, NROUND, 2, ROWS, W], f16, isOutput=True)

    with TileContext(nc) as tc:
        with tc.tile_pool(name="xp", bufs=1) as xpool, \
             tc.tile_pool(name="const", bufs=1) as cpool, \
             tc.tile_pool(name="stage", bufs=3) as opool, \
             tc.tile_pool(name="psum", bufs=8, space="PSUM") as ppool:

            xp = xpool.tile([128, Hp, Wp], f16)
            wt = cpool.tile([128, NTAP, Cout], f16)
            bt = cpool.tile([128, 1], f32)
            zt = cpool.tile([32, 512], f16)

            # x chunk 0 first: its completion receipt gates the first
            # real matmul, so it must land earliest
            bounds = [0, 11, 43, 75, 107, Hp]
            nc.sync.dma_start(out=xp[:, 0:bounds[1], :],
                              in_=x_ext[:, 0:bounds[1], :])
            nc.sync.dma_start(out=wt[:], in_=w_ext[:])
            nc.sync.dma_start(out=bt[:], in_=b_ext[:])
            for g in range(1, len(bounds) - 1):
                a, b = bounds[g], bounds[g + 1]
                nc.sync.dma_start(out=xp[:, a:b, :], in_=x_ext[:, a:b, :])

            # PE warm-up: release the HAM clock gate during the x load.
            nc.vector.memset(zt[:].bitcast(f32), 0.0)
            pw = ppool.tile([128, ROWS, W], f32, tag="ps", name="warm")
            for i in range(NWARM):
                nc.tensor.matmul(pw[0:64], zt[0:32, 0:64], zt[0:32, :],
                                 start=True, stop=True, tile_position=(0, 0),
                                 skip_group_check=True)

            for (k0, k1) in GROUPS:
                ost = opool.tile([128, k1 - k0, 2, ROWS, W], f16, tag="ost")
                for k in range(k0, k1):
                    h0 = k * ROWS
                    slot = k - k0
                    ps0 = ppool.tile([128, ROWS, W], f32, tag="ps",
                                     name=f"ps{k}_0")   # imgs 0,2
                    ps1 = ppool.tile([128, ROWS, W], f32, tag="ps",
                                     name=f"ps{k}_1")   # imgs 1,3
                    for t in range(NTAP):
                        kh, kw = divmod(t, 3)
                        st, sp = (t == 0), (t == NTAP - 1)
                        rows = xp[:, h0 + kh:h0 + kh + ROWS, kw:kw + W]
                        nc.tensor.matmul(ps0[0:64], wt[0:32, t, :],
                                         rows[0:32], start=st, stop=sp,
                                         tile_position=(0, 0))
                        nc.tensor.matmul(ps1[0:64], wt[32:64, t, :],
                                         rows[32:64], start=st, stop=sp,
                                         tile_position=(32, 0))
                        nc.tensor.matmul(ps0[64:128], wt[64:96, t, :],
                                         rows[64:96], start=st, stop=sp,
                                         tile_position=(64, 64))
                        nc.tensor.matmul(ps1[64:128], wt[96:128, t, :],
                                         rows[96:128], start=st, stop=sp,
                                         tile_position=(96, 64))
                    # full-width drains: +bias, cast f32->f16
                    nc.scalar.activation(ost[:, slot, 0, :, :], ps0[:, :, :],
                                         Act.Identity, bias=bt[:, :])
                    nc.vector.tensor_scalar_add(ost[:, slot, 1, :, :],
                                                ps1[:, :, :], bt[:, :])
                if k1 - k0 > 1:
                    nc.sync.dma_start(out=out_ext[:, k0:k1], in_=ost[:])
                else:
                    # final rounds: store each pair as soon as its drain lands
                    nc.sync.dma_start(out=out_ext[:, k0:k1, 0],
                                      in_=ost[:, :, 0])
                    nc.sync.dma_start(out=out_ext[:, k0:k1, 1],
                                      in_=ost[:, :, 1])

    nc.compile()
    return nc


def _get_program():
    if "nc" not in _cache:
        _cache["nc"] = _build_program()
    return _cache["nc"]


def _prep_inputs(x, kernel, bias):
    # weights: (Cout, Cin, KH, KW) -> [ci, tap, co], replicated on the
    # 4 PE row groups
    w = np.transpose(kernel.reshape(Cout, Cin, NTAP), (1, 2, 0))
    w = np.ascontiguousarray(np.tile(w, (4, 1, 1))).astype(np.float16)
    b = np.ascontiguousarray(
        np.tile(bias.astype(np.float32), 2)[:, None])
    x16 = x.astype(np.float16)
    in_maps = []
    for c in range(NCORES):
        xs = np.zeros((128, Hp, Wp), dtype=np.float16)
        xs[:, 1:1 + H, 1:1 + W] = x16[c * BPC:(c + 1) * BPC].reshape(
            BPC * Cin, H, W)
        in_maps.append({"x": xs, "w": w, "b": b})
    return in_maps


def _unshard(res):
    outs = []
    for c in range(NCORES):
        a = res.results[c]["out"]          # [128, NROUND, 2, ROWS, W] f16
        a = a.reshape(2, Cout, NROUND, 2, ROWS, W)   # [ph, co, k, pair, rho, w]
        a = np.transpose(a, (0, 3, 1, 2, 4, 5))      # [ph, pair, co, k, rho, w]
        outs.append(a.reshape(BPC, Cout, H, W))
    return np.concatenate(outs, axis=0).astype(np.float32)


def _run(inputs, trace=False):
    from concourse.bass_utils import run_bass_kernel_spmd
    nc = _get_program()
    in_maps = _prep_inputs(inputs["x"], inputs["kernel"], inputs["bias"])
    res = run_bass_kernel_spmd(nc, in_maps, list(range(NCORES)), trace=trace)
    return _unshard(res), res


def kernel(**inputs):
    out, _ = _run(inputs, trace=False)
    return out
